# revision 11
# baseline (speedup 1.0000x reference)
"""AttnBlock (GroupNorm + spatial self-attention + proj + residual) on 8 TRN2 cores.

Problem shapes (hardcoded): x (4, 512, 64, 64) fp32, 1x1-conv weights (512, 512).

Sharding: 8 cores = (batch b in 0..3) x (query half qh in 0..1). Attention is
permutation-invariant over key positions, so each core receives its batch's
x rotated along the flattened spatial axis so that its own 2048 query
positions are always columns 0:2048 -- the compiled NEFF is identical on all
cores (pure SPMD, no collectives).

Compute strategy (fp8 DoubleRow): the four large contractions (merged-qk
conv, v conv, scores, attention-weighted v) run as fp8e4 matmuls with
perf_mode=DoubleRow (contracting 256 rows/pass). The final proj conv stays
fp16: its quantization error lands directly on the output and does not
average out over keys. All PSUM accumulation is fp32. Softmax: scores for
query block i are s = h_k . q'_i with q' = (Wq^T Wk)^T h + Wk^T bq (the
bias fold makes nonzero bq/bk exact; per-query terms cancel in softmax).
exp(scale*s - 3) goes PE->ACT->fp8; the softmax denominator is accumulated
on the PE itself (an all-ones fp8 DoubleRow matmul per key-tile pair into a
dedicated PSUM bank, giving the replicated denominator for free), and
divides the attention numerator *before* the proj conv (division commutes
with the channel contraction). v bias is folded as bp' = Wp @ bv + bp on
the host.

x stays resident in SBUF for the residual (no re-load). ~96 back-to-back
warm-up matmuls on one scratch bank run while x streams in so the PE's HAM
clock-gate reaches 8/8 before the first conv matmul. PSUM: 3 rotating
score banks + 1 denominator bank + 4 attention accumulator banks.
"""

from contextlib import ExitStack

import numpy as np
import ml_dtypes

import concourse.bacc as bacc
import concourse.mybir as mybir
import concourse.tile as tile
from concourse.bass_utils import run_bass_kernel_spmd

F32 = mybir.dt.float32
F16 = mybir.dt.float16
F8 = mybir.dt.float8e4
DR = mybir.MatmulPerfMode.DoubleRow
NP_F8 = ml_dtypes.float8_e4m3

C = 512          # channels
N = 4096         # spatial positions (64*64)
NQ = 2048        # query positions per core
P = 128          # partitions
CT = C // P      # 4 channel tiles
NB = 512         # matmul free-dim block
NJ = N // P      # 32 key tiles
NJP = NJ // 2    # 16 key tile pairs
G = 32           # groups
GS = C // G      # 16 channels per group
GPT = P // GS    # 8 groups per channel tile
EPS = 1e-6
SCALE = float(C) ** -0.5
EXP_BIAS = -3.0  # constant max-proxy; cancels in the softmax ratio
WARM_MM = 96     # HAM warm-up matmuls during the x DMA

N_CORES = 8


def _emit(ctx: ExitStack, tc: tile.TileContext):
    nc = tc.nc
    x_d = nc.declare_dram_parameter("x", [C, N], F32, isOutput=False)
    wm_d = nc.declare_dram_parameter("wm", [P, CT, C], F8, isOutput=False)
    wv_d = nc.declare_dram_parameter("wv", [P, CT, C], F8, isOutput=False)
    wp_d = nc.declare_dram_parameter("wp", [P, CT, C], F16, isOutput=False)
    bqc_d = nc.declare_dram_parameter("bqc", [C], F32, isOutput=False)
    bp2_d = nc.declare_dram_parameter("bp2", [C], F32, isOutput=False)
    gamma_d = nc.declare_dram_parameter("gamma", [C], F32, isOutput=False)
    beta_d = nc.declare_dram_parameter("beta", [C], F32, isOutput=False)
    mask_d = nc.declare_dram_parameter("gmask", [P, GPT], F32, isOutput=False)
    expand_d = nc.declare_dram_parameter("gexpand", [GPT, P], F32, isOutput=False)
    out_d = nc.declare_dram_parameter("out", [C, NQ], F32, isOutput=True)

    consts = ctx.enter_context(tc.tile_pool(name="consts", bufs=1))
    big = ctx.enter_context(tc.tile_pool(name="big", bufs=1))
    gn_small = ctx.enter_context(tc.tile_pool(name="gn_small", bufs=2))
    exp_pool = ctx.enter_context(tc.tile_pool(name="exp_pool", bufs=2))
    att_sb_pool = ctx.enter_context(tc.tile_pool(name="att_sb_pool", bufs=2))
    out_pool = ctx.enter_context(tc.tile_pool(name="out_pool", bufs=2))
    ps_sc = ctx.enter_context(tc.tile_pool(name="ps_sc", bufs=3, space="PSUM"))
    ps_den = ctx.enter_context(tc.tile_pool(name="ps_den", bufs=1, space="PSUM"))
    ps_att = ctx.enter_context(tc.tile_pool(name="ps_att", bufs=1, space="PSUM"))

    ident_f = mybir.ActivationFunctionType.Identity
    exp_f = mybir.ActivationFunctionType.Exp

    # ---- small constants via memset (no DMA dependency) ----
    ones16 = consts.tile([P, P], F16, name="ones16", tag="ones16")
    nc.vector.memset(ones16, 1.0)
    ones8 = consts.tile([P, 2, P], F8, name="ones8", tag="ones8")
    nc.vector.memset(ones8, 1.0)
    onef = consts.tile([P, 1], F32, name="onef", tag="onef")
    nc.vector.memset(onef, 1.0)
    expbias_sb = consts.tile([P, 1], F32, name="expbias_sb", tag="expbias_sb")
    nc.vector.memset(expbias_sb, EXP_BIAS)

    # ---- HAM warm-up: dense back-to-back matmuls on one scratch bank ----
    wt = ps_sc.tile([P, NB], F32, name="warm", tag="mm")
    for w in range(WARM_MM):
        nc.tensor.matmul(wt[:, :P], lhsT=ones16, rhs=ones16,
                         start=True, stop=True)

    def warm_fill(nm, n=4):
        # tiny no-dependency matmuls into the (pre-attention) den bank; they
        # bridge PE idle gaps in the DMA-paced head so HAM stays at 8/8
        wf = ps_den.tile([P, NB], F32, name=nm, tag="den")
        for _ in range(n):
            nc.tensor.matmul(wf[:, :P], lhsT=ones16, rhs=ones16,
                             start=True, stop=True)

    # ---- x stream on the HWDGE (sync) queue; weights interleaved so they
    # land before first use; small constants via SWDGE (gpsimd) ----
    xs = big.tile([P, CT, N], F32, name="xs", tag="xs")
    for t in range(CT):
        for ch in range(4):
            nc.sync.dma_start(out=xs[:, t, ch * (N // 4):(ch + 1) * (N // 4)],
                              in_=x_d[t * P:(t + 1) * P,
                                      ch * (N // 4):(ch + 1) * (N // 4)])
        if t == 1:
            wm_sb = consts.tile([P, CT, C], F8, name="wm_sb", tag="wm_sb")
            nc.sync.dma_start(out=wm_sb, in_=wm_d[:, :, :])
            wv_sb = consts.tile([P, CT, C], F8, name="wv_sb", tag="wv_sb")
            nc.sync.dma_start(out=wv_sb, in_=wv_d[:, :, :])
        if t == 2:
            wp_sb = consts.tile([P, CT, C], F16, name="wp_sb", tag="wp_sb")
            nc.sync.dma_start(out=wp_sb, in_=wp_d[:, :, :])

    mask_sb = consts.tile([P, GPT], F32, name="mask_sb", tag="mask_sb")
    nc.gpsimd.dma_start(out=mask_sb, in_=mask_d[:, :])
    expand_sb = consts.tile([GPT, P], F32, name="expand_sb", tag="expand_sb")
    nc.gpsimd.dma_start(out=expand_sb, in_=expand_d[:, :])

    def load_vec(ap, nm):
        r = ap[:].rearrange("(t p) -> t p", p=P)
        tiles = []
        for t in range(CT):
            tl = consts.tile([P, 1], F32, name=f"{nm}_{t}", tag=f"{nm}_{t}")
            nc.gpsimd.dma_start(out=tl, in_=r[t][:, None])
            tiles.append(tl)
        return tiles

    gamma_sb = load_vec(gamma_d, "gamma")
    beta_sb = load_vec(beta_d, "beta")
    bqc_sb = load_vec(bqc_d, "bqc")
    bp2_sb = load_vec(bp2_d, "bp2")

    # ---- persistent big tensors ----
    h8 = big.tile([P, CT, N], F8, name="h8", tag="h8")
    q8 = big.tile([P, CT, NQ], F8, name="q8", tag="q8")
    vt8 = big.tile([P, NJ, C], F8, name="vt8", tag="vt8")

    # GN small-matmul scratch uses the (pre-attention) att PSUM banks
    gn_ps = ps_att.tile([P, CT, NB], F32, name="gn_ps", tag="att")

    # ---- phase 1: GroupNorm -> h8 (fp8) ----
    # All bn_stats are emitted first so the (FIFO) vector queue is never
    # blocked behind a later tile's cross-engine round trips; the per-tile
    # chains keep DVE out of the middle (copies on ACT, reciprocal on
    # gpsimd's exact divide).
    st_tiles = []
    for t in range(CT):
        st = gn_small.tile([P, N // NB, 6], F32, name=f"st_{t}", tag=f"st{t}")
        xs_c = xs[:, t, :].rearrange("p (c f) -> p c f", f=NB)
        for cchunk in range(N // NB):
            nc.vector.bn_stats(out=st[:, cchunk, :], in_=xs_c[:, cchunk, :])
        st_tiles.append(st)
    vpe_t, gmv_t, rv_t = [], [], []
    for t in range(CT):
        ms2 = gn_small.tile([P, 2], F32, name=f"ms2_{t}", tag="ms2")
        nc.vector.bn_aggr(out=ms2, in_=st_tiles[t])
        msq = gn_small.tile([P, 1], F32, name=f"msq_{t}", tag="msq")
        nc.gpsimd.tensor_tensor(msq, ms2[:, 0:1], ms2[:, 0:1],
                                mybir.AluOpType.mult)
        nc.gpsimd.tensor_add(ms2[:, 1:2], ms2[:, 1:2], msq)
        # group-average across the 16-channel partition runs: mask matmul
        nc.tensor.matmul(gn_ps[:GPT, t, 0:2], lhsT=mask_sb, rhs=ms2,
                         start=True, stop=True)
        gmv = gn_small.tile([GPT, 2], F32, name=f"gmv_{t}", tag=f"gmv{t}")
        nc.scalar.copy(out=gmv, in_=gn_ps[:GPT, t, 0:2])
        vpe = gn_small.tile([GPT, 1], F32, name=f"vpe_{t}", tag=f"vpe{t}")
        nc.gpsimd.tensor_tensor(vpe, gmv[:, 0:1], gmv[:, 0:1],
                                mybir.AluOpType.mult)
        nc.gpsimd.tensor_scalar(vpe, gmv[:, 1:2], vpe, EPS,
                                mybir.AluOpType.subtract, mybir.AluOpType.add)
        vpe_t.append(vpe)
        gmv_t.append(gmv)
    for t in range(CT):
        # rstd = sqrt(1/(var+eps)); rstd error is dominated by the fp8 h
        rv = gn_small.tile([GPT, 1], F32, name=f"rv_{t}", tag=f"rv{t}")
        nc.vector.reciprocal(out=rv, in_=vpe_t[t])
        rv_t.append(rv)
    for t in range(CT):
        grs = gn_small.tile([GPT, 2], F32, name=f"grs_{t}", tag="grs")
        nc.gpsimd.tensor_copy(out=grs[:, 0:1], in_=gmv_t[t][:, 0:1])
        nc.scalar.sqrt(out=grs[:, 1:2], in_=rv_t[t])
        nc.tensor.matmul(gn_ps[:, t, 2:4], lhsT=expand_sb, rhs=grs,
                         start=True, stop=True)
        cms = gn_small.tile([P, 2], F32, name=f"cms_{t}", tag="cms")
        nc.scalar.copy(out=cms, in_=gn_ps[:, t, 2:4])
        a_t = gn_small.tile([P, 1], F32, name=f"a_{t}", tag="a")
        nc.gpsimd.tensor_tensor(a_t, gamma_sb[t], cms[:, 1:2],
                                mybir.AluOpType.mult)
        b_t = gn_small.tile([P, 1], F32, name=f"b_{t}", tag="b")
        nc.gpsimd.tensor_tensor(b_t, cms[:, 0:1], a_t, mybir.AluOpType.mult)
        nc.gpsimd.tensor_tensor(b_t, beta_sb[t], b_t, mybir.AluOpType.subtract)
        # h8 = fp8(x*A + B) -- split across ACT and DVE
        nc.scalar.activation(out=h8[:, t, :N // 2], in_=xs[:, t, :N // 2],
                             func=ident_f, bias=b_t, scale=a_t)
        nc.vector.tensor_scalar(h8[:, t, N // 2:], xs[:, t, N // 2:], a_t, b_t,
                                mybir.AluOpType.mult, mybir.AluOpType.add)
        warm_fill(f"wgn_{t}", 6)

    # ---- phase 2: q' and vT convs (fp8 DoubleRow, single-bank groups) ----
    # PSUM rotates over ps_sc's 3 banks plus the (idle) attention banks;
    # PSUM->fp8 copies alternate between ACT and DVE so neither engine
    # paces the PE.
    conv_n = 0
    cur_att = [None]

    def conv_psum(nm):
        nonlocal conv_n
        r = conv_n % 7
        conv_n += 1
        if r < 3:
            return ps_sc.tile([P, NB], F32, name=nm, tag="mm")
        if r == 3:
            cur_att[0] = ps_att.tile([P, CT, NB], F32, name=nm, tag="att")
        return cur_att[0][:, r - 3, :]

    for nb in range(NQ // NB):
        for co in range(CT):
            ps = conv_psum(f"qps_{co}_{nb}")
            for half in range(2):
                nc.tensor.matmul(
                    ps,
                    lhsT=wm_sb[:, 2 * half:2 * half + 2, co * P:(co + 1) * P],
                    rhs=h8[:, 2 * half:2 * half + 2, nb * NB:(nb + 1) * NB],
                    start=(half == 0), stop=(half == 1), perf_mode=DR)
            qv = q8[:, co, nb * NB:(nb + 1) * NB]
            if (co + nb) % 2 == 0:
                nc.scalar.activation(out=qv, in_=ps, func=ident_f,
                                     bias=bqc_sb[co], scale=1.0)
            else:
                nc.vector.tensor_scalar_add(qv, ps, bqc_sb[co])
        warm_fill(f"wq_{nb}", 4)

    for j in range(NJ):
        ps = conv_psum(f"vps_{j}")
        for half in range(2):
            nc.tensor.matmul(
                ps,
                lhsT=h8[:, 2 * half:2 * half + 2, j * P:(j + 1) * P],
                rhs=wv_sb[:, 2 * half:2 * half + 2, :],
                start=(half == 0), stop=(half == 1), perf_mode=DR)
        if j % 2 == 0:
            nc.scalar.copy(out=vt8[:, j, :], in_=ps)
        else:
            nc.vector.tensor_copy(out=vt8[:, j, :], in_=ps)
        if j % 4 == 3:
            warm_fill(f"wv_{j}", 4)

    # ---- phase 3: attention + proj + epilogue, per query block ----
    def emit_tail(ib, att_ps, den_ps, xpb, chunks=1):
        # chunks>1 pipelines the (otherwise serial) final-block epilogue in
        # narrow column slices so the output DMA starts early
        cw = NB // chunks
        rb = out_pool.tile([P, NB], F32, name=f"rb_{ib}", tag="rb")
        rscr = out_pool.tile([P, NB], F32, name=f"rscr_{ib}", tag="rscr")
        att_sb = att_sb_pool.tile([P, CT, NB], F16, name=f"asb_{ib}",
                                  tag="asb")
        pp = ps_att.tile([P, CT, NB], F32, name=f"pp_{ib}", tag="att")
        for ck in range(chunks):
            cs = slice(ck * cw, (ck + 1) * cw)
            nc.vector.reciprocal_approx_accurate(out=rb[:, cs],
                                                 in_=den_ps[:, cs],
                                                 scratch=rscr[:, cs])
            # normalize the attention numerator before proj (fp16)
            for c in range(CT):
                nc.vector.tensor_tensor(att_sb[:, c, cs], att_ps[:, c, cs],
                                        rb[:, cs], mybir.AluOpType.mult)
            # proj into the freed attention banks (att_sb writes precede in
            # program order; the next block's first att matmul follows)
            for co in range(CT):
                for ci in range(CT):
                    nc.tensor.matmul(pp[:, co, cs],
                                     lhsT=wp_sb[:, ci, co * P:(co + 1) * P],
                                     rhs=att_sb[:, ci, cs],
                                     start=(ci == 0), stop=(ci == CT - 1))
            for co in range(CT):
                fin = out_pool.tile([P, cw], F32, name=f"fin_{ib}_{co}_{ck}",
                                    tag=f"fin{co}")
                nc.vector.tensor_tensor(fin, pp[:, co, cs],
                                        xpb[:, co, cs],
                                        mybir.AluOpType.add)
                nc.sync.dma_start(
                    out=out_d[co * P:(co + 1) * P,
                              ib * NB + ck * cw:ib * NB + (ck + 1) * cw],
                    in_=fin)

    pending = None
    for ib in range(NQ // NB):
        isl = slice(ib * NB, (ib + 1) * NB)
        ex_t = exp_pool.tile([P, NJ, NB], F8, name=f"ex_{ib}", tag="ex")
        att_ps = den_ps = None
        xpb = out_pool.tile([P, CT, NB], F32, name=f"xpb_{ib}", tag="xpb")
        for step in range(NJP + 1):
            if step < NJP:
                for jj in range(2):
                    j = step * 2 + jj
                    sc = ps_sc.tile([P, NB], F32, name=f"sc_{ib}_{j}",
                                    tag="mm")
                    for half in range(2):
                        nc.tensor.matmul(
                            sc,
                            lhsT=h8[:, 2 * half:2 * half + 2,
                                    j * P:(j + 1) * P],
                            rhs=q8[:, 2 * half:2 * half + 2, isl],
                            start=(half == 0), stop=(half == 1), perf_mode=DR)
                    nc.scalar.activation(out=ex_t[:, j, :], in_=sc,
                                         func=exp_f,
                                         bias=expbias_sb, scale=SCALE)
            if pending is not None and step == 1:
                emit_tail(*pending)
                pending = None
            if step == 2:
                # residual + folded proj bias, ready before the epilogue
                for co in range(CT):
                    nc.vector.tensor_scalar_add(xpb[:, co, :],
                                                xs[:, co, isl], bp2_sb[co])
            if step >= 1:
                sp = step - 1
                if sp == 0:
                    att_ps = ps_att.tile([P, CT, NB], F32,
                                         name=f"attps_{ib}", tag="att")
                    den_ps = ps_den.tile([P, NB], F32, name=f"den_{ib}",
                                         tag="den")
                exv = ex_t[:, sp * 2:sp * 2 + 2, :]
                for c in range(CT):
                    nc.tensor.matmul(
                        att_ps[:, c, :],
                        lhsT=vt8[:, sp * 2:sp * 2 + 2, c * P:(c + 1) * P],
                        rhs=exv,
                        start=(sp == 0), stop=(sp == NJP - 1), perf_mode=DR)
                nc.tensor.matmul(den_ps, lhsT=ones8, rhs=exv,
                                 start=(sp == 0), stop=(sp == NJP - 1),
                                 perf_mode=DR)
        pending = (ib, att_ps, den_ps, xpb)
    emit_tail(*pending, chunks=4)


_CACHED = {}


def _build(merged=True):
    if "nc" not in _CACHED:
        nc = bacc.Bacc()
        with tile.TileContext(nc) as tc, ExitStack() as ctx:
            _emit(ctx, tc)
        nc.finalize()
        _CACHED["nc"] = nc
    return _CACHED["nc"]


def _host_inputs(x, norm_gamma, norm_beta, Wq, bq, Wk, bk, Wv, bv, Wp, bp,
                 merged=None):
    Wq64 = np.asarray(Wq, np.float64)
    Wk64 = np.asarray(Wk, np.float64)
    wm = (Wq64.T @ Wk64).astype(np.float32)          # q' = wm^T h (+ bqc)
    bqc = (Wk64.T @ np.asarray(bq, np.float64)).astype(np.float32)

    def pack(w, dt):
        # [ci, co] -> [P, CT, C] with ci = t*128 + p
        return np.ascontiguousarray(
            np.asarray(w, np.float32).reshape(CT, P, C).transpose(1, 0, 2)
        ).astype(dt)

    common = {
        "wm": pack(wm, NP_F8),
        "wv": pack(np.asarray(Wv, np.float32).T, NP_F8),
        "wp": pack(np.asarray(Wp, np.float32).T, np.float16),
        "bqc": bqc,
        "bp2": (np.asarray(Wp, np.float64) @ np.asarray(bv, np.float64)
                + np.asarray(bp, np.float64)).astype(np.float32),
        "gamma": np.asarray(norm_gamma, np.float32),
        "beta": np.asarray(norm_beta, np.float32),
        "gmask": ((np.arange(P)[:, None] // GS == np.arange(GPT)[None, :])
                  .astype(np.float32) / GS),
        "gexpand": (np.arange(GPT)[:, None] == np.arange(P)[None, :] // GS)
                   .astype(np.float32),
    }
    xf = np.asarray(x, np.float32).reshape(4, C, N)
    in_maps = []
    for core in range(N_CORES):
        bi, qh = core // 2, core % 2
        xc = np.ascontiguousarray(np.roll(xf[bi], -qh * NQ, axis=1))
        in_maps.append({"x": xc, **common})
    return in_maps


def kernel(x, norm_gamma, norm_beta, Wq, bq, Wk, bk, Wv, bv, Wp, bp):
    x = np.asarray(x, np.float32)
    b, c, hh, ww = x.shape
    assert (b, c, hh * ww) == (4, C, N)
    nc = _build()
    in_maps = _host_inputs(x, norm_gamma, norm_beta,
                           Wq, bq, Wk, bk, Wv, bv, Wp, bp)
    res = run_bass_kernel_spmd(nc, in_maps, core_ids=list(range(N_CORES)))
    y = np.empty((4, C, N), np.float32)
    for core in range(N_CORES):
        bi, qh = core // 2, core % 2
        y[bi][:, qh * NQ:(qh + 1) * NQ] = res.results[core]["out"]
    return y.reshape(b, c, hh, ww)


# revision 15
# speedup vs baseline: 1.1996x; 1.1996x over previous
"""AttnBlock (GroupNorm + spatial self-attention + proj + residual) on 8 TRN2 cores.

Problem shapes (hardcoded): x (4, 512, 64, 64) fp32, 1x1-conv weights (512, 512).

Sharding: 8 cores = (batch b in 0..3) x (query half qh in 0..1). Attention is
permutation-invariant over key positions, so each core receives its batch's
x rotated along the flattened spatial axis so that its own 2048 query
positions are always columns 0:2048 -- the compiled NEFF is identical on all
cores (pure SPMD, no collectives).

Compute strategy (fp8 DoubleRow): the four large contractions (merged-qk
conv, v conv, scores, attention-weighted v) run as fp8e4 matmuls with
perf_mode=DoubleRow (contracting 256 rows/pass). The final proj conv stays
fp16: its quantization error lands directly on the output and does not
average out over keys. All PSUM accumulation is fp32. Softmax: scores for
query block i are s = h_k . q'_i with q' = (Wq^T Wk)^T h + Wk^T bq (the
bias fold makes nonzero bq/bk exact; per-query terms cancel in softmax).
exp(scale*s - 3) goes PE->ACT->fp8; the softmax denominator is accumulated
on the PE itself (an all-ones fp8 DoubleRow matmul per key-tile pair into a
dedicated PSUM bank, giving the replicated denominator for free), and
divides the attention numerator *before* the proj conv (division commutes
with the channel contraction). v bias is folded as bp' = Wp @ bv + bp on
the host.

x stays resident in SBUF for the residual (no re-load). ~96 back-to-back
warm-up matmuls on one scratch bank run while x streams in so the PE's HAM
clock-gate reaches 8/8 before the first conv matmul. PSUM: 3 rotating
score banks + 1 denominator bank + 4 attention accumulator banks.
"""

from contextlib import ExitStack

import numpy as np
import ml_dtypes

import concourse.bacc as bacc
import concourse.mybir as mybir
import concourse.tile as tile
from concourse.bass_utils import run_bass_kernel_spmd

F32 = mybir.dt.float32
F16 = mybir.dt.float16
F8 = mybir.dt.float8e4
DR = mybir.MatmulPerfMode.DoubleRow
NP_F8 = ml_dtypes.float8_e4m3

C = 512          # channels
N = 4096         # spatial positions (64*64)
NQ = 2048        # query positions per core
P = 128          # partitions
CT = C // P      # 4 channel tiles
NB = 512         # matmul free-dim block
NJ = N // P      # 32 key tiles
NJP = NJ // 2    # 16 key tile pairs
G = 32           # groups
GS = C // G      # 16 channels per group
GPT = P // GS    # 8 groups per channel tile
EPS = 1e-6
SCALE = float(C) ** -0.5
EXP_BIAS = -3.0  # constant max-proxy; cancels in the softmax ratio
WARM_MM = 96     # HAM warm-up matmuls during the x DMA

N_CORES = 8


def _emit(ctx: ExitStack, tc: tile.TileContext):
    nc = tc.nc
    x_d = nc.declare_dram_parameter("x", [C, N], F32, isOutput=False)
    wm_d = nc.declare_dram_parameter("wm", [P, CT, C], F8, isOutput=False)
    wv_d = nc.declare_dram_parameter("wv", [P, CT, C], F8, isOutput=False)
    wp_d = nc.declare_dram_parameter("wp", [P, CT, C], F16, isOutput=False)
    bqc_d = nc.declare_dram_parameter("bqc", [C], F32, isOutput=False)
    bp2_d = nc.declare_dram_parameter("bp2", [C], F32, isOutput=False)
    gamma_d = nc.declare_dram_parameter("gamma", [C], F32, isOutput=False)
    beta_d = nc.declare_dram_parameter("beta", [C], F32, isOutput=False)
    mask_d = nc.declare_dram_parameter("gmask", [P, GPT], F32, isOutput=False)
    expand_d = nc.declare_dram_parameter("gexpand", [GPT, P], F32, isOutput=False)
    out_d = nc.declare_dram_parameter("out", [C, NQ], F32, isOutput=True)

    consts = ctx.enter_context(tc.tile_pool(name="consts", bufs=1))
    big = ctx.enter_context(tc.tile_pool(name="big", bufs=1))
    gn_small = ctx.enter_context(tc.tile_pool(name="gn_small", bufs=2))
    exp_pool = ctx.enter_context(tc.tile_pool(name="exp_pool", bufs=2))
    att_sb_pool = ctx.enter_context(tc.tile_pool(name="att_sb_pool", bufs=2))
    out_pool = ctx.enter_context(tc.tile_pool(name="out_pool", bufs=2))
    ps_sc = ctx.enter_context(tc.tile_pool(name="ps_sc", bufs=3, space="PSUM"))
    ps_den = ctx.enter_context(tc.tile_pool(name="ps_den", bufs=1, space="PSUM"))
    ps_att = ctx.enter_context(tc.tile_pool(name="ps_att", bufs=1, space="PSUM"))

    ident_f = mybir.ActivationFunctionType.Identity
    exp_f = mybir.ActivationFunctionType.Exp

    # ---- small constants via memset (no DMA dependency) ----
    ones16 = consts.tile([P, P], F16, name="ones16", tag="ones16")
    nc.vector.memset(ones16, 1.0)
    ones8 = consts.tile([P, 2, P], F8, name="ones8", tag="ones8")
    nc.vector.memset(ones8, 1.0)
    onef = consts.tile([P, 1], F32, name="onef", tag="onef")
    nc.vector.memset(onef, 1.0)
    expbias_sb = consts.tile([P, 1], F32, name="expbias_sb", tag="expbias_sb")
    nc.vector.memset(expbias_sb, EXP_BIAS)

    # ---- HAM warm-up: dense back-to-back matmuls on one scratch bank ----
    wt = ps_sc.tile([P, NB], F32, name="warm", tag="mm")
    for w in range(WARM_MM):
        nc.tensor.matmul(wt[:, :P], lhsT=ones16, rhs=ones16,
                         start=True, stop=True)

    def warm_fill(nm, n=4):
        # tiny no-dependency matmuls into the (pre-attention) den bank; they
        # bridge PE idle gaps in the DMA-paced head so HAM stays at 8/8
        wf = ps_den.tile([P, NB], F32, name=nm, tag="den")
        for _ in range(n):
            nc.tensor.matmul(wf[:, :P], lhsT=ones16, rhs=ones16,
                             start=True, stop=True)

    # ---- x stream on the HWDGE (sync) queue; weights interleaved so they
    # land before first use; small constants via SWDGE (gpsimd) ----
    xs = big.tile([P, CT, N], F32, name="xs", tag="xs")
    for t in range(CT):
        for ch in range(4):
            nc.sync.dma_start(out=xs[:, t, ch * (N // 4):(ch + 1) * (N // 4)],
                              in_=x_d[t * P:(t + 1) * P,
                                      ch * (N // 4):(ch + 1) * (N // 4)])
        if t == 1:
            wm_sb = consts.tile([P, CT, C], F8, name="wm_sb", tag="wm_sb")
            nc.sync.dma_start(out=wm_sb, in_=wm_d[:, :, :])
            wv_sb = consts.tile([P, CT, C], F8, name="wv_sb", tag="wv_sb")
            nc.sync.dma_start(out=wv_sb, in_=wv_d[:, :, :])
        if t == 2:
            wp_sb = consts.tile([P, CT, C], F16, name="wp_sb", tag="wp_sb")
            nc.sync.dma_start(out=wp_sb, in_=wp_d[:, :, :])

    mask_sb = consts.tile([P, GPT], F32, name="mask_sb", tag="mask_sb")
    nc.gpsimd.dma_start(out=mask_sb, in_=mask_d[:, :])
    expand_sb = consts.tile([GPT, P], F32, name="expand_sb", tag="expand_sb")
    nc.gpsimd.dma_start(out=expand_sb, in_=expand_d[:, :])

    def load_vec(ap, nm):
        r = ap[:].rearrange("(t p) -> t p", p=P)
        tiles = []
        for t in range(CT):
            tl = consts.tile([P, 1], F32, name=f"{nm}_{t}", tag=f"{nm}_{t}")
            nc.gpsimd.dma_start(out=tl, in_=r[t][:, None])
            tiles.append(tl)
        return tiles

    gamma_sb = load_vec(gamma_d, "gamma")
    beta_sb = load_vec(beta_d, "beta")
    bqc_sb = load_vec(bqc_d, "bqc")
    bp2_sb = load_vec(bp2_d, "bp2")

    # ---- persistent big tensors ----
    h8 = big.tile([P, CT, N], F8, name="h8", tag="h8")
    q8 = big.tile([P, CT, NQ], F8, name="q8", tag="q8")
    vt8 = big.tile([P, NJ, C], F8, name="vt8", tag="vt8")

    # GN small-matmul scratch uses the (pre-attention) att PSUM banks
    gn_ps = ps_att.tile([P, CT, NB], F32, name="gn_ps", tag="att")

    # ---- phase 1: GroupNorm -> h8 (fp8) ----
    # Per-channel sums: DVE chunk-reduces Sum(x) while ACT's Square+accum_out
    # produces Sum(x^2) (no bn_stats -- halves the DVE head work). The
    # 1/(GS*N) group averaging is folded into the host-side mask, so the
    # mask matmul directly yields [group mean, group E[x^2]]. The PE queue
    # is in-order, so the mask/expand matmuls are padded with warm-fill
    # matmuls sized under the expected stats wait (keeps HAM at 8/8).
    ms2_t = []
    sqscr = gn_small.tile([P, N // 4], F16, name="sqscr", tag="sqscr")
    for t in range(CT):
        sx = gn_small.tile([P, 2, 4], F32, name=f"sx_{t}", tag=f"sx{t}")
        for cchunk in range(4):
            cs = slice(cchunk * (N // 4), (cchunk + 1) * (N // 4))
            nc.vector.tensor_reduce(out=sx[:, 0, cchunk:cchunk + 1],
                                    in_=xs[:, t, cs],
                                    axis=mybir.AxisListType.X,
                                    op=mybir.AluOpType.add)
            nc.scalar.activation(out=sqscr, in_=xs[:, t, cs],
                                 func=mybir.ActivationFunctionType.Square,
                                 accum_out=sx[:, 1, cchunk:cchunk + 1])
        ms2 = gn_small.tile([P, 2], F32, name=f"ms2_{t}", tag=f"ms2{t}")
        nc.vector.tensor_reduce(out=ms2, in_=sx,
                                axis=mybir.AxisListType.X,
                                op=mybir.AluOpType.add)
        ms2_t.append(ms2)
    for t in range(CT):
        warm_fill(f"wgn_{t}", 30 if t else 50)
        # group-average across the 16-channel partition runs: mask matmul
        nc.tensor.matmul(gn_ps[:GPT, t, 0:2], lhsT=mask_sb, rhs=ms2_t[t],
                         start=True, stop=True)
        gmv = gn_small.tile([GPT, 2], F32, name=f"gmv_{t}", tag=f"gmv{t}")
        nc.scalar.copy(out=gmv, in_=gn_ps[:GPT, t, 0:2])
        vpe = gn_small.tile([GPT, 1], F32, name=f"vpe_{t}", tag=f"vpe{t}")
        nc.gpsimd.tensor_tensor(vpe, gmv[:, 0:1], gmv[:, 0:1],
                                mybir.AluOpType.mult)
        nc.gpsimd.tensor_scalar(vpe, gmv[:, 1:2], vpe, EPS,
                                mybir.AluOpType.subtract, mybir.AluOpType.add)
        # rstd = sqrt(1/(var+eps)); rstd error is dominated by the fp8 h
        rv = gn_small.tile([GPT, 1], F32, name=f"rv_{t}", tag=f"rv{t}")
        nc.vector.reciprocal(out=rv, in_=vpe)
        grs = gn_small.tile([GPT, 2], F32, name=f"grs_{t}", tag="grs")
        nc.gpsimd.tensor_copy(out=grs[:, 0:1], in_=gmv[:, 0:1])
        nc.scalar.sqrt(out=grs[:, 1:2], in_=rv)
        warm_fill(f"wge_{t}", 6)
        nc.tensor.matmul(gn_ps[:, t, 2:4], lhsT=expand_sb, rhs=grs,
                         start=True, stop=True)
        cms = gn_small.tile([P, 2], F32, name=f"cms_{t}", tag="cms")
        nc.scalar.copy(out=cms, in_=gn_ps[:, t, 2:4])
        a_t = gn_small.tile([P, 1], F32, name=f"a_{t}", tag="a")
        nc.gpsimd.tensor_tensor(a_t, gamma_sb[t], cms[:, 1:2],
                                mybir.AluOpType.mult)
        b_t = gn_small.tile([P, 1], F32, name=f"b_{t}", tag="b")
        nc.gpsimd.tensor_tensor(b_t, cms[:, 0:1], a_t, mybir.AluOpType.mult)
        nc.gpsimd.tensor_tensor(b_t, beta_sb[t], b_t, mybir.AluOpType.subtract)
        # h8 = fp8(x*A + B) -- split across ACT and DVE
        nc.scalar.activation(out=h8[:, t, :N // 2], in_=xs[:, t, :N // 2],
                             func=ident_f, bias=b_t, scale=a_t)
        nc.vector.tensor_scalar(h8[:, t, N // 2:], xs[:, t, N // 2:], a_t, b_t,
                                mybir.AluOpType.mult, mybir.AluOpType.add)

    # ---- phase 2: q' and vT convs (fp8 DoubleRow, single-bank groups) ----
    # PSUM rotates over ps_sc's 3 banks plus the (idle) attention banks;
    # PSUM->fp8 copies alternate between ACT and DVE so neither engine
    # paces the PE.
    conv_n = 0
    cur_att = [None]

    def conv_psum(nm):
        nonlocal conv_n
        r = conv_n % 7
        conv_n += 1
        if r < 3:
            return ps_sc.tile([P, NB], F32, name=nm, tag="mm")
        if r == 3:
            cur_att[0] = ps_att.tile([P, CT, NB], F32, name=nm, tag="att")
        return cur_att[0][:, r - 3, :]

    for nb in range(NQ // NB):
        for co in range(CT):
            ps = conv_psum(f"qps_{co}_{nb}")
            for half in range(2):
                nc.tensor.matmul(
                    ps,
                    lhsT=wm_sb[:, 2 * half:2 * half + 2, co * P:(co + 1) * P],
                    rhs=h8[:, 2 * half:2 * half + 2, nb * NB:(nb + 1) * NB],
                    start=(half == 0), stop=(half == 1), perf_mode=DR)
            qv = q8[:, co, nb * NB:(nb + 1) * NB]
            if (co + nb) % 2 == 0:
                nc.scalar.activation(out=qv, in_=ps, func=ident_f,
                                     bias=bqc_sb[co], scale=1.0)
            else:
                nc.vector.tensor_scalar_add(qv, ps, bqc_sb[co])

    for j in range(NJ):
        ps = conv_psum(f"vps_{j}")
        for half in range(2):
            nc.tensor.matmul(
                ps,
                lhsT=h8[:, 2 * half:2 * half + 2, j * P:(j + 1) * P],
                rhs=wv_sb[:, 2 * half:2 * half + 2, :],
                start=(half == 0), stop=(half == 1), perf_mode=DR)
        if j % 2 == 0:
            nc.scalar.copy(out=vt8[:, j, :], in_=ps)
        else:
            nc.vector.tensor_copy(out=vt8[:, j, :], in_=ps)

    # ---- phase 3: attention + proj + epilogue, per query block ----
    def emit_tail(ib, att_ps, den_ps, xpb, chunks=1):
        # chunks>1 pipelines the (otherwise serial) final-block epilogue in
        # narrow column slices so the output DMA starts early
        cw = NB // chunks
        rb = out_pool.tile([P, NB], F32, name=f"rb_{ib}", tag="rb")
        rscr = out_pool.tile([P, NB], F32, name=f"rscr_{ib}", tag="rscr")
        att_sb = att_sb_pool.tile([P, CT, NB], F16, name=f"asb_{ib}",
                                  tag="asb")
        pp = ps_att.tile([P, CT, NB], F32, name=f"pp_{ib}", tag="att")
        for ck in range(chunks):
            cs = slice(ck * cw, (ck + 1) * cw)
            nc.vector.reciprocal_approx_accurate(out=rb[:, cs],
                                                 in_=den_ps[:, cs],
                                                 scratch=rscr[:, cs])
            # normalize the attention numerator before proj (fp16)
            for c in range(CT):
                nc.vector.tensor_tensor(att_sb[:, c, cs], att_ps[:, c, cs],
                                        rb[:, cs], mybir.AluOpType.mult)
            # proj into the freed attention banks (att_sb writes precede in
            # program order; the next block's first att matmul follows)
            for co in range(CT):
                for ci in range(CT):
                    nc.tensor.matmul(pp[:, co, cs],
                                     lhsT=wp_sb[:, ci, co * P:(co + 1) * P],
                                     rhs=att_sb[:, ci, cs],
                                     start=(ci == 0), stop=(ci == CT - 1))
            for co in range(CT):
                fin = out_pool.tile([P, cw], F32, name=f"fin_{ib}_{co}_{ck}",
                                    tag=f"fin{co}")
                nc.vector.tensor_tensor(fin, pp[:, co, cs],
                                        xpb[:, co, cs],
                                        mybir.AluOpType.add)
                nc.sync.dma_start(
                    out=out_d[co * P:(co + 1) * P,
                              ib * NB + ck * cw:ib * NB + (ck + 1) * cw],
                    in_=fin)

    pending = None
    for ib in range(NQ // NB):
        isl = slice(ib * NB, (ib + 1) * NB)
        ex_t = exp_pool.tile([P, NJ, NB], F8, name=f"ex_{ib}", tag="ex")
        att_ps = den_ps = None
        xpb = out_pool.tile([P, CT, NB], F32, name=f"xpb_{ib}", tag="xpb")
        for step in range(NJP + 1):
            if step < NJP:
                for jj in range(2):
                    j = step * 2 + jj
                    sc = ps_sc.tile([P, NB], F32, name=f"sc_{ib}_{j}",
                                    tag="mm")
                    for half in range(2):
                        nc.tensor.matmul(
                            sc,
                            lhsT=h8[:, 2 * half:2 * half + 2,
                                    j * P:(j + 1) * P],
                            rhs=q8[:, 2 * half:2 * half + 2, isl],
                            start=(half == 0), stop=(half == 1), perf_mode=DR)
                    nc.scalar.activation(out=ex_t[:, j, :], in_=sc,
                                         func=exp_f,
                                         bias=expbias_sb, scale=SCALE)
            if pending is not None and step == 1:
                emit_tail(*pending)
                pending = None
            if step == 2:
                # residual + folded proj bias, ready before the epilogue
                for co in range(CT):
                    nc.vector.tensor_scalar_add(xpb[:, co, :],
                                                xs[:, co, isl], bp2_sb[co])
            if step >= 1:
                sp = step - 1
                if sp == 0:
                    att_ps = ps_att.tile([P, CT, NB], F32,
                                         name=f"attps_{ib}", tag="att")
                    den_ps = ps_den.tile([P, NB], F32, name=f"den_{ib}",
                                         tag="den")
                exv = ex_t[:, sp * 2:sp * 2 + 2, :]
                for c in range(CT):
                    nc.tensor.matmul(
                        att_ps[:, c, :],
                        lhsT=vt8[:, sp * 2:sp * 2 + 2, c * P:(c + 1) * P],
                        rhs=exv,
                        start=(sp == 0), stop=(sp == NJP - 1), perf_mode=DR)
                nc.tensor.matmul(den_ps, lhsT=ones8, rhs=exv,
                                 start=(sp == 0), stop=(sp == NJP - 1),
                                 perf_mode=DR)
        pending = (ib, att_ps, den_ps, xpb)
    emit_tail(*pending, chunks=4)


_CACHED = {}


def _build(merged=True):
    if "nc" not in _CACHED:
        nc = bacc.Bacc()
        with tile.TileContext(nc) as tc, ExitStack() as ctx:
            _emit(ctx, tc)
        nc.finalize()
        _CACHED["nc"] = nc
    return _CACHED["nc"]


def _host_inputs(x, norm_gamma, norm_beta, Wq, bq, Wk, bk, Wv, bv, Wp, bp,
                 merged=None):
    Wq64 = np.asarray(Wq, np.float64)
    Wk64 = np.asarray(Wk, np.float64)
    wm = (Wq64.T @ Wk64).astype(np.float32)          # q' = wm^T h (+ bqc)
    bqc = (Wk64.T @ np.asarray(bq, np.float64)).astype(np.float32)

    def pack(w, dt):
        # [ci, co] -> [P, CT, C] with ci = t*128 + p
        return np.ascontiguousarray(
            np.asarray(w, np.float32).reshape(CT, P, C).transpose(1, 0, 2)
        ).astype(dt)

    common = {
        "wm": pack(wm, NP_F8),
        "wv": pack(np.asarray(Wv, np.float32).T, NP_F8),
        "wp": pack(np.asarray(Wp, np.float32).T, np.float16),
        "bqc": bqc,
        "bp2": (np.asarray(Wp, np.float64) @ np.asarray(bv, np.float64)
                + np.asarray(bp, np.float64)).astype(np.float32),
        "gamma": np.asarray(norm_gamma, np.float32),
        "beta": np.asarray(norm_beta, np.float32),
        "gmask": ((np.arange(P)[:, None] // GS == np.arange(GPT)[None, :])
                  .astype(np.float32) / (GS * N)),
        "gexpand": (np.arange(GPT)[:, None] == np.arange(P)[None, :] // GS)
                   .astype(np.float32),
    }
    xf = np.asarray(x, np.float32).reshape(4, C, N)
    in_maps = []
    for core in range(N_CORES):
        bi, qh = core // 2, core % 2
        xc = np.ascontiguousarray(np.roll(xf[bi], -qh * NQ, axis=1))
        in_maps.append({"x": xc, **common})
    return in_maps


def kernel(x, norm_gamma, norm_beta, Wq, bq, Wk, bk, Wv, bv, Wp, bp):
    x = np.asarray(x, np.float32)
    b, c, hh, ww = x.shape
    assert (b, c, hh * ww) == (4, C, N)
    nc = _build()
    in_maps = _host_inputs(x, norm_gamma, norm_beta,
                           Wq, bq, Wk, bk, Wv, bv, Wp, bp)
    res = run_bass_kernel_spmd(nc, in_maps, core_ids=list(range(N_CORES)))
    y = np.empty((4, C, N), np.float32)
    for core in range(N_CORES):
        bi, qh = core // 2, core % 2
        y[bi][:, qh * NQ:(qh + 1) * NQ] = res.results[core]["out"]
    return y.reshape(b, c, hh, ww)
